# revision 57
# baseline (speedup 1.0000x reference)
"""Trainium2 Bass kernel for nn_MultiHeadAttention_43971875177057.

MHA with residual: B=2, S=4096, d_model=512, n_heads=8, dk=64.
out = (softmax(mask(QK^T/sqrt(dk))) @ V).reshape(b,s,d) @ Wo^T + bo + x
where the reshape interleaves heads and sequence (no transpose back).

Sharding: 8 cores = (batch b in {0,1}) x (head-pair hp in {0..3}).
Each core computes attention for 2 heads of one batch over the full
sequence.

v3 design notes:
 * exp() is mostly a Schraudolph bit-trick fused with the mask into ONE
   DVE op: Q is pre-scaled by A=2^7/ln2, so bits_i16=(psum+16256)*m01,
   reinterpreted as bf16.  The remaining exp stays on ACT (with
   scale=1/A) and its mask-multiply goes to GPSIMD/DVE.  This splits the
   old 278us serial ACT EXP across three engines.
 * QC=512 so score psums are single-bank: double-buffered per head
   (4 banks) + qc-parity ctx psums (4 banks) = 8.  Scores matmuls then
   wait on elementwise from 2 kt ago (long done), so the two heads'
   64-contraction matmuls are ready together and issue back-to-back as
   concurrent PE row-tiles (h0 rows 0-63, h1 rows 64-127).
 * Softmax sums ride the ones-column of V; the epilogue copies raw ctx
   to SBUF immediately, bounces sums through DRAM for a 128-lane
   reciprocal, normalizes at 2x bf16 rate off the critical path.  The
   normalized ones-row (==1.0) folds the output bias into the
   out-projection as a 65th contraction row.
 * Output + residual are bf16 (host upcasts); out-proj runs as a tail
   when all PSUM banks are free.
"""

import os
import sys
import types

import numpy as np
import ml_dtypes

B, S, D, H, DK = 2, 4096, 512, 8, 64
QC = 512           # q-chunk (free dim of score tiles)
RT = S // 8        # output rows per head (the interleaved reshape)
BF16 = ml_dtypes.bfloat16

A_SCH = 128.0 / 0.6931471805599453   # 2^7 / ln 2
B_KEEP = 16256.0                     # 127 << 7  (bf16 exponent bias)
CTXLAG = 3


def _build_kernel(n_cores=8):
    import concourse.bacc as bacc
    import concourse.mybir as mybir
    import concourse.tile as tile
    import concourse.bass as bass

    f32 = mybir.dt.float32
    bf16 = mybir.dt.bfloat16
    i16 = mybir.dt.int16
    C = D // 128       # 4 contraction chunks for the projections
    NKT = S // 128     # 32 key tiles
    NQC = S // QC      # 8 q chunks
    NKQ = NKT // NQC   # 4 key tiles per q-chunk span

    nc = bacc.Bacc("TRN2", target_bir_lowering=False, debug=False,
                   num_devices=n_cores)

    xT = nc.dram_tensor("xT", [D, S], bf16, kind="ExternalInput").ap()
    maskT = nc.dram_tensor("maskT", [S, S], bf16, kind="ExternalInput").ap()
    wq = nc.dram_tensor("wq", [128, C * 128], bf16, kind="ExternalInput").ap()
    wk = nc.dram_tensor("wk", [128, C * 128], bf16, kind="ExternalInput").ap()
    wv = nc.dram_tensor("wv", [128, C * 128], bf16, kind="ExternalInput").ap()
    wo = nc.dram_tensor("wo", [128, 8 * D], bf16, kind="ExternalInput").ap()
    bqs = nc.dram_tensor("bqs", [128, 1], f32, kind="ExternalInput").ap()
    bks = nc.dram_tensor("bks", [128, 1], f32, kind="ExternalInput").ap()
    bv = nc.dram_tensor("bv", [1, 128], f32, kind="ExternalInput").ap()
    xres = nc.dram_tensor("xres", [2 * RT, D], bf16, kind="ExternalInput").ap()
    out = nc.dram_tensor("out", [2 * RT, D], bf16, kind="ExternalOutput").ap()
    sum_dram = nc.dram_tensor("sum_scratch", [2 * NQC, QC], f32).ap()
    rc_dram = nc.dram_tensor("rc_scratch", [2 * NQC, QC], bf16).ap()

    Exp = mybir.ActivationFunctionType.Exp
    Copy = mybir.ActivationFunctionType.Copy
    Mul = mybir.AluOpType.mult
    Add = mybir.AluOpType.add

    def pbcast(ap, p):
        # broadcast a [1, ...] DRAM AP along partitions
        return bass.AP(tensor=ap.tensor, offset=ap.offset,
                       ap=[[0, p]] + list(ap.ap[1:]))

    with tile.TileContext(nc) as tc:
        with (
            tc.tile_pool(name="const", bufs=1) as const,
            tc.tile_pool(name="mask", bufs=12) as maskp,
            tc.tile_pool(name="attn", bufs=10) as attnp,
            tc.tile_pool(name="araw", bufs=4) as arawp,
            tc.tile_pool(name="outp", bufs=2) as outp,
            tc.tile_pool(name="small", bufs=2) as small,
            tc.tile_pool(name="psum", bufs=1, space="PSUM") as psum,
        ):
            # ---- loads ------------------------------------------------
            xT_r = xT.rearrange("(c p) s -> c p s", p=128)
            xt_c = [const.tile([128, S], bf16, tag=f"xt{c}", name=f"xt{c}")
                    for c in range(C)]
            for quarter in range(4):
                lo, hi = quarter * (S // 4), (quarter + 1) * (S // 4)
                for c in range(C):
                    nc.sync.dma_start(out=xt_c[c][:, lo:hi],
                                      in_=xT_r[c][:, lo:hi])
            wq_sb = const.tile([128, C, 128], bf16)
            nc.sync.dma_start(out=wq_sb, in_=wq.rearrange("p (c n) -> p c n", c=C))
            wk_sb = const.tile([128, C, 128], bf16)
            nc.sync.dma_start(out=wk_sb, in_=wk.rearrange("p (c n) -> p c n", c=C))
            wv_sb = const.tile([128, C, 128], bf16)
            nc.sync.dma_start(out=wv_sb, in_=wv.rearrange("p (c n) -> p c n", c=C))
            wo_sb = const.tile([128, 8, D], bf16)
            nc.sync.dma_start(out=wo_sb, in_=wo.rearrange("c (j f) -> c j f", j=8))
            bq_sb = const.tile([128, 1], f32)
            nc.sync.dma_start(out=bq_sb, in_=bqs)
            bk_sb = const.tile([128, 1], f32)
            nc.sync.dma_start(out=bk_sb, in_=bks)
            bv_sb = const.tile([128, 128], f32)
            nc.sync.dma_start(out=bv_sb, in_=pbcast(bv, 128))
            ones_sb = const.tile([128, 1], bf16)
            nc.vector.memset(ones_sb, 1.0)

            # ---- mask prefetch ---------------------------------------
            # One [128, 2*QC] tile per (qc-pair, kt) serves both in-flight
            # q-chunks -- halves the descriptor-gen load on the sync queue.
            units = [(qcp, kt) for qcp in range(NQC // 2) for kt in range(NKT)]
            mask_tiles = {}

            def load_mask(ui):
                if ui >= len(units):
                    return
                qcp, kt = units[ui]
                mt = maskp.tile([128, 2 * QC], bf16, tag="mt")
                nc.sync.dma_start(
                    out=mt,
                    in_=maskT[kt * 128:kt * 128 + 128,
                              qcp * 2 * QC:(qcp + 1) * 2 * QC])
                mask_tiles[ui] = mt

            # ---- projections (serial prologue) ------------------------
            qt_c = [None] * NQC
            kt_c = [None] * NQC
            v_c = [const.tile([128, NKQ, 128], bf16, tag=f"v{i}", name=f"v{i}")
                   for i in range(NQC)]

            def proj_qk(nm, w_sb, b_sb, lst, scale, i):
                ps = psum.tile([128, QC], f32,
                               tag="s00" if nm == "qt" else "s10", name="pqk")
                for c in range(C):
                    nc.tensor.matmul(ps, lhsT=w_sb[:, c, :],
                                     rhs=xt_c[c][:, i * QC:(i + 1) * QC],
                                     start=(c == 0), stop=(c == C - 1))
                t = const.tile([128, QC], bf16, tag=f"{nm}{i}", name=f"{nm}{i}")
                nc.vector.tensor_scalar(t, ps, scale, b_sb, Mul, Add)
                lst[i] = t

            def proj_v(kt):
                ps = psum.tile([128, 128], f32, tag="s11", name="pv")
                for c in range(C):
                    nc.tensor.matmul(ps, lhsT=xt_c[c][:, kt * 128:(kt + 1) * 128],
                                     rhs=wv_sb[:, c, :],
                                     start=(c == 0), stop=(c == C - 1))
                nc.vector.tensor_add(v_c[kt // NKQ][:, kt % NKQ, :], ps, bv_sb)

            for i in range(NQC):
                proj_qk("qt", wq_sb, bq_sb, qt_c, 0.125 * A_SCH, i)
                proj_qk("kt", wk_sb, bk_sb, kt_c, 1.0, i)
                for kt in range(i * NKQ, (i + 1) * NKQ):
                    proj_v(kt)

            for ui in range(6):
                load_mask(ui)

            # ---- attention --------------------------------------------
            # ctx for both heads shares one [128, S] layout: h0 rows 0-63,
            # h1 rows 64-127 (matches the col-tiled ctx psum).
            ctxT = const.tile([128, S], bf16, tag="ctxT", name="ctxT")
            ctxN = const.tile([128, S], bf16, tag="ctxN", name="ctxN")

            def emit_scores(qc, kt, qch):
                k0 = kt * 128
                kq = kt_c[k0 // QC]
                kk = k0 % QC
                sps = [psum.tile([128, QC], f32, tag=f"s{h}{qch}",
                                 name=f"sps{h}") for h in (0, 1)]
                for h in (0, 1):
                    nc.tensor.matmul(
                        sps[h],
                        lhsT=kq[h * 64:(h + 1) * 64, kk:kk + 128],
                        rhs=qt_c[qc][h * 64:(h + 1) * 64, :],
                        start=True, stop=True)
                return sps

            def emit_elementwise(mt, kt, qch, sps):
                # Q is pre-scaled by A_SCH host-side, so the scores psum is
                # already in Schraudolph code units: bits = (psum + B) * m01.
                ms = mt[:, qch * QC:(qch + 1) * QC]
                ats = []
                # h0: fused Schraudolph+mask on DVE; ACT exp 1-in-8
                at0 = attnp.tile([128, QC], bf16, tag=f"a0{qch}")
                if kt % 4 != 3:
                    nc.vector.scalar_tensor_tensor(
                        at0[:, :].bitcast(i16), sps[0], B_KEEP, ms, Add, Mul)
                else:
                    ar0 = arawp.tile([128, QC], bf16, tag=f"ar0{qch}")
                    nc.scalar.activation(ar0, sps[0], Exp, scale=1.0 / A_SCH)
                    nc.vector.tensor_mul(at0, ar0, ms)
                ats.append(at0)
                # h1: ACT exp, mask-mul split between gpsimd and DVE
                ar1 = arawp.tile([128, QC], bf16, tag=f"ar1{qch}")
                nc.scalar.activation(ar1, sps[1], Exp, scale=1.0 / A_SCH)
                at1 = attnp.tile([128, QC], bf16, tag=f"a1{qch}")
                eng = nc.vector if (2 * kt + qch) % 3 == 1 else nc.gpsimd
                eng.tensor_mul(at1, ar1, ms)
                ats.append(at1)
                return ats

            def emit_ctx(kt, qch, ats, ctx_ps, sum_ps):
                # col-tiled pair: h0 -> psum cols 0-63, h1 -> 64-127; then a
                # concurrent ones-vector pair for the softmax sums.
                for h in (0, 1):
                    nc.tensor.matmul(
                        ctx_ps[qch][h * 64:(h + 1) * 64, :],
                        lhsT=v_c[kt // NKQ][:, kt % NKQ, h * 64:(h + 1) * 64],
                        rhs=ats[h],
                        start=(kt == 0), stop=(kt == NKT - 1))
                for h in (0, 1):
                    nc.tensor.matmul(
                        sum_ps[qch][h * 32:h * 32 + 1, :],
                        lhsT=ones_sb,
                        rhs=ats[h],
                        start=(kt == 0), stop=(kt == NKT - 1))

            for qcp in range(NQC // 2):
                ctx_ps = [psum.tile([128, QC], f32, tag=f"c{qch}",
                                    name=f"ctx{qch}") for qch in (0, 1)]
                sum_ps = [psum.tile([33, QC], f32, tag=f"m{qch}",
                                    name=f"sum{qch}") for qch in (0, 1)]
                at_hist = {}
                for kt in range(NKT):
                    ui = qcp * NKT + kt
                    load_mask(ui + 6)
                    mt = mask_tiles.pop(ui)
                    for qch in (0, 1):
                        sps = emit_scores(2 * qcp + qch, kt, qch)
                        if kt >= CTXLAG and qch == 0:
                            for q2 in (0, 1):
                                emit_ctx(kt - CTXLAG, q2,
                                         at_hist.pop((kt - CTXLAG, q2)),
                                         ctx_ps, sum_ps)
                        at_hist[(kt, qch)] = emit_elementwise(mt, kt, qch, sps)
                for kt in range(NKT - CTXLAG, NKT):
                    for q2 in (0, 1):
                        emit_ctx(kt, q2, at_hist.pop((kt, q2)), ctx_ps, sum_ps)

                # ---- per-qc epilogue: free psum fast, normalize later --
                for qch in (0, 1):
                    qc = 2 * qcp + qch
                    q0 = qc * QC
                    scp = small.tile([33, QC], f32, tag="scp")
                    nc.scalar.activation(scp, sum_ps[qch], Copy)
                    nc.vector.tensor_copy(ctxT[:, q0:q0 + QC], ctx_ps[qch])
                    sraw = sum_dram[qc * 2:qc * 2 + 2, :]
                    nc.sync.dma_start(out=sraw[0:1, :], in_=scp[0:1, :])
                    nc.sync.dma_start(out=sraw[1:2, :], in_=scp[32:33, :])
                    sums = small.tile([128, QC // 64], f32, tag="sums")
                    nc.sync.dma_start(
                        out=sums,
                        in_=sraw.rearrange("o (p f) -> (o p) f", p=64))
                    rc = small.tile([128, QC // 64], bf16, tag="rc")
                    with nc.allow_low_precision(reason="softmax recip"):
                        nc.vector.reciprocal(rc, sums)
                    rows = rc_dram[qc * 2:qc * 2 + 2, :]
                    nc.sync.dma_start(
                        out=rows.rearrange("o (p f) -> (o p) f", p=64),
                        in_=rc)
                    rcr = small.tile([128, QC], bf16, tag="rcr")
                    nc.sync.dma_start(out=rcr[0:64, :],
                                      in_=pbcast(rows[0:1, :], 64))
                    nc.sync.dma_start(out=rcr[64:128, :],
                                      in_=pbcast(rows[1:2, :], 64))
                    nc.vector.tensor_mul(ctxN[:, q0:q0 + QC],
                                         ctxT[:, q0:q0 + QC], rcr)

            # ---- output projection tail ------------------------------
            # row-tiled concurrent pairs: h0 contraction rows 0-63, h1 64-127
            # (wo is host-duplicated into both partition halves).  bo is
            # pre-added into xres host-side.
            ctx3 = ctxN.rearrange("p (t j) -> p j t", j=8)
            for qcp in range(4):
                ops = [psum.tile([128, D], f32, tag=f"s{h}0", name=f"ops{h}")
                       for h in (0, 1)]
                for j in range(8):
                    for h in (0, 1):
                        nc.tensor.matmul(
                            ops[h],
                            lhsT=ctx3[h * 64:(h + 1) * 64, j,
                                      qcp * 128:(qcp + 1) * 128],
                            rhs=wo_sb[h * 64:(h + 1) * 64, j, :],
                            start=(j == 0), stop=(j == 7))
                for h in (0, 1):
                    r0 = h * RT + qcp * 128
                    xr = outp.tile([128, D], bf16, tag="xr")
                    nc.sync.dma_start(out=xr, in_=xres[r0:r0 + 128, :])
                    osb = outp.tile([128, D], bf16, tag="osb")
                    nc.vector.tensor_add(osb, ops[h], xr)
                    nc.sync.dma_start(out=out[r0:r0 + 128, :], in_=osb)

    nc.compile()
    return nc


def _shard_inputs(x, mask, Wq, bq, Wk, bk, Wv, bv, Wo, bo):
    """Host-side marshaling: slice/transpose/cast per core. core = b*4+hp."""
    C = D // 128
    keepT = np.ascontiguousarray((1 - mask[0, 0]).T).astype(BF16)
    woT = Wo.T.astype(np.float32)
    wo_half = woT.reshape(8, 64, D).transpose(1, 0, 2)   # [64, 8, D]
    wo_re = np.ascontiguousarray(
        np.concatenate([wo_half, wo_half], axis=0).reshape(128, 8 * D)
    ).astype(BF16)

    def re_w(wT):
        # [D, n] -> [128, C*n]  with  out[p, c*n+j] = wT[c*128+p, j]
        n = wT.shape[1]
        return np.ascontiguousarray(
            wT.reshape(C, 128, n).transpose(1, 0, 2).reshape(128, C * n)
        ).astype(BF16)

    in_maps = []
    for core in range(8):
        b, hp = divmod(core, 4)
        c0 = hp * 128
        in_maps.append({
            "xT": np.ascontiguousarray(x[b].T).astype(BF16),
            "maskT": keepT,
            "wq": re_w(np.ascontiguousarray(Wq[c0:c0 + 128, :].T)),
            "wk": re_w(np.ascontiguousarray(Wk[c0:c0 + 128, :].T)),
            "wv": re_w(np.ascontiguousarray(Wv[c0:c0 + 128, :].T)),
            "wo": wo_re,
            "bqs": (bq[c0:c0 + 128] * (A_SCH / 8.0)
                    ).reshape(128, 1).astype(np.float32),
            "bks": bk[c0:c0 + 128].reshape(128, 1).astype(np.float32),
            "bv": bv[c0:c0 + 128].reshape(1, 128).astype(np.float32),
            "xres": np.ascontiguousarray(
                x[b, hp * 2 * RT:(hp + 1) * 2 * RT, :] + bo).astype(BF16),
        })
    return in_maps


_RESULT_CACHE = {}


def _ensure_env():
    """Make concourse importable and register the NTFF profile hook."""
    for p in ("/root/.axon_site/_ro/trn_rl_repo", "/opt/trn_rl_repo"):
        if os.path.isdir(p) and p not in sys.path:
            sys.path.append(p)
    try:
        import antenv  # noqa: F401
        import antenv.axon_hooks  # noqa: F401
    except ImportError:
        try:
            import antenv
            mod = types.ModuleType("antenv.axon_hooks")
            _hook = [None]
            mod.set_axon_ntff_profile_hook = lambda h: _hook.__setitem__(0, h)
            mod.get_axon_ntff_profile_hook = lambda: _hook[0]
            sys.modules["antenv.axon_hooks"] = mod
            antenv.axon_hooks = mod
            from trn_agent_boot.trn_boot import _ntff_profile_via_ctypes
            so = "/opt/axon/libaxon_pjrt.so"
            if os.path.exists(so):
                mod.set_axon_ntff_profile_hook(_ntff_profile_via_ctypes(so))
        except Exception:
            pass


def kernel(x, mask, Wq, bq, Wk, bk, Wv, bv, Wo, bo, trace=False):
    _ensure_env()
    from concourse.bass_utils import run_bass_kernel_spmd

    x = np.asarray(x, np.float32)
    mask = np.asarray(mask)
    args = [np.asarray(a, np.float32) for a in (Wq, bq, Wk, bk, Wv, bv, Wo, bo)]
    nc = _RESULT_CACHE.get("nc")
    if nc is None:
        nc = _build_kernel()
        _RESULT_CACHE["nc"] = nc
    in_maps = _shard_inputs(x, mask, *args)
    res = run_bass_kernel_spmd(nc, in_maps, core_ids=list(range(8)),
                               trace=trace)
    _RESULT_CACHE["last_run"] = res
    out = np.empty((B, S, D), np.float32)
    for core in range(8):
        b, hp = divmod(core, 4)
        out[b, hp * 2 * RT:(hp + 1) * 2 * RT, :] = \
            res.results[core]["out"].astype(np.float32)
    return out


if __name__ == "__main__":
    _ensure_env()
    nc = _build_kernel()
    print("kernel built + compiled OK")
